# revision 3
# baseline (speedup 1.0000x reference)
"""
DLI loss kernel for Trainium2 (8 NeuronCores, pure data parallel over batch).

Math
----
The reference computes, per (b, j) window pair:
    logits[b,j,k] = h_last[b,j]@w_h + cterm[b,k] + fc_b
    loss_pair     = LSE_k(logits masked to k in [j+3, len_b)) - logits[b,j,j+3]
The h_last@w_h and fc_b terms are constant in k, so they cancel exactly
between the LSE and the positive logit.  The whole LSTM drops out and

    loss = sum_{b, s in [3, len_b)} [ log(sum_{k=s}^{len_b-1} e^{cterm[b,k]})
                                      - cterm[b,s] ] / sum_b (len_b - 3)
    cterm[b,k] = encoder_output[b,k,:] @ fc_w[0, H:]   (valid region only)

cterm values are O(+-2) so no max-subtraction is needed for a stable exp.

Device pipeline (per core, 16 batch rows)
-----------------------------------------
The host marshals enc into a per-core [E, BPC*T] layout in bf16 (the same
round-to-nearest cast the previous device pipeline applied on-chip), so E
lands on partitions and each DMA chunk is 128 clean 4 KB descriptors.  It
also pre-expands fc_w into the one-hot matvec weight matrix woh
(woh[e, 16b + m] = w[e] * (m == b)).

  - 4 enc DMAs (4 rows each) on the sync HWDGE queue; woh + mask ride the
    scalar queue.
  - 16 accumulating PE matvecs (lhsT = woh column block, rhs = enc slice)
    compute cterm for all 16 rows into one PSUM tile [16, 512] - no
    transposes, no casts, no PSUM->SBUF round trips.
  - Tail: exp (ACT, fused time-reverse via the PSUM read AP),
    tensor_tensor_scan (suffix sums with mask fold), Ln(x+1) with
    accumulate, masked-cterm accumulate, and a tiny PE reduction to
    [numer, denom].
  - Host sums the 8 per-core [numer, denom] pairs and divides.
"""

import ml_dtypes
import numpy as np

import concourse.bacc as bacc
import concourse.bass as bass
import concourse.mybir as mybir
import concourse.tile as tile
from concourse._compat import with_exitstack
from concourse.bass_utils import run_bass_kernel_spmd

B, T, E, H = 128, 512, 128, 128
NCORES = 8
BPC = B // NCORES  # batch rows per core
RPD = 4  # rows per DMA chunk
NCHUNK = BPC // RPD

f32 = mybir.dt.float32
bf16 = mybir.dt.bfloat16
i32 = mybir.dt.int32


@with_exitstack
def _dli_body(ctx, tc):
    nc = tc.nc

    enc = nc.dram_tensor("enc", [E, BPC * T], bf16, kind="ExternalInput").ap()
    woh_d = nc.dram_tensor("woh", [E, BPC * BPC], bf16, kind="ExternalInput").ap()
    msk = nc.dram_tensor("mask", [BPC, T], i32, kind="ExternalInput").ap()
    out = nc.dram_tensor("out", [2], f32, kind="ExternalOutput").ap()

    const_pool = ctx.enter_context(tc.tile_pool(name="const", bufs=1))
    ct_psum = ctx.enter_context(tc.tile_pool(name="ct_psum", bufs=1, space="PSUM"))
    fin_psum = ctx.enter_context(tc.tile_pool(name="fin_psum", bufs=1, space="PSUM"))
    sc_pool = ctx.enter_context(tc.tile_pool(name="scan", bufs=1))

    # woh and mask ride the (otherwise idle) scalar HWDGE queue so the
    # matvecs and the tail never wait behind the enc stream.
    woh = const_pool.tile([E, BPC * BPC], bf16)
    nc.scalar.dma_start(woh[:], woh_d[:, :])
    msk_sb = sc_pool.tile([BPC, T], i32)
    nc.scalar.dma_start(msk_sb[:], msk[:, :])

    # enc stream: 4 rows per DMA (128 x 4 KB descriptors), sync queue.
    enc_sb = const_pool.tile([E, BPC * T], bf16)
    CW = RPD * T
    for q in range(NCHUNK):
        nc.sync.dma_start(
            enc_sb[:, q * CW : (q + 1) * CW], enc[:, q * CW : (q + 1) * CW]
        )

    # cterm for all 16 rows accumulated in one PSUM tile; free index = t.
    cterm_ps = ct_psum.tile([BPC, T], f32)
    for b in range(BPC):
        nc.tensor.matmul(
            cterm_ps[:, :],
            lhsT=woh[:, BPC * b : BPC * (b + 1)],
            rhs=enc_sb[:, b * T : (b + 1) * T],
            start=(b == 0),
            stop=(b == BPC - 1),
        )

    # mask -> f32, zero first 3 time steps (window starts need s >= 3)
    maskf = sc_pool.tile([BPC, T], f32)
    nc.vector.tensor_copy(maskf[:], msk_sb[:])
    nc.vector.memset(maskf[:, 0:3], 0.0)
    mask3_rev = maskf[:, ::-1]

    # denominator: sum(mask3) = len_b - 3 (mask-only, runs during the stream)
    acc = sc_pool.tile([BPC, 2], f32)
    packed = sc_pool.tile([BPC, 2], f32)
    nc.vector.tensor_reduce(
        packed[:, 1:2], maskf[:], axis=mybir.AxisListType.X, op=mybir.AluOpType.add
    )
    ones = const_pool.tile([BPC, 1], f32)
    nc.vector.memset(ones[:], 1.0)

    # E = exp(cterm), time-reversed via the PSUM read AP
    e_sb = sc_pool.tile([BPC, T], f32)
    nc.scalar.activation(e_sb[:], cterm_ps[:, ::-1], mybir.ActivationFunctionType.Exp)

    # suffix sums with the mask folded into the scan:
    # state = (E[i] + state) * mask3_rev[i] - resets across the invalid
    # tail, accumulates sum(exp) over the valid region.
    s_sb = sc_pool.tile([BPC, T], f32)
    nc.vector.tensor_tensor_scan(
        s_sb[:], e_sb[:], mask3_rev, 0.0, mybir.AluOpType.add, mybir.AluOpType.mult
    )

    # u = (S - 1) * mask3; then ln(u + 1) = log(S) on valid, 0 on invalid
    u_sb = sc_pool.tile([BPC, T], f32)
    nc.vector.scalar_tensor_tensor(
        u_sb[:], s_sb[:], 1.0, mask3_rev,
        mybir.AluOpType.subtract, mybir.AluOpType.mult,
    )
    ln_sb = sc_pool.tile([BPC, T], f32)
    nc.scalar.activation(
        ln_sb[:], u_sb[:], mybir.ActivationFunctionType.Ln,
        bias=1.0, scale=1.0, accum_out=acc[:, 0:1],
    )

    # sum(mask3*cterm): needs only the matvec results; emitted after the
    # scan/stt so it overlaps the ACT Ln instead of blocking the DVE chain.
    mc_sb = sc_pool.tile([BPC, T], f32)
    nc.vector.scalar_tensor_tensor(
        mc_sb[:], cterm_ps[:, :], 0.0, maskf[:],
        mybir.AluOpType.add, mybir.AluOpType.mult, accum_out=acc[:, 1:2],
    )

    # packed[:,0] = numer_b = sum(ln) - sum(mask3*cterm)
    nc.vector.tensor_tensor(
        packed[:, 0:1], acc[:, 0:1], acc[:, 1:2], mybir.AluOpType.subtract
    )

    # cross-partition reduce on PE: out = packed^T @ ones = [sum numer, sum denom]
    fin = fin_psum.tile([2, 1], f32)
    nc.tensor.matmul(fin[:, :], lhsT=packed[:, :], rhs=ones[:, :], start=True, stop=True)
    out_sb = sc_pool.tile([2, 1], f32)
    nc.vector.tensor_copy(out_sb[:], fin[:, :])
    nc.scalar.dma_start(out.rearrange("(p one) -> p one", one=1), out_sb[:])


_CACHED_NC = None


def _get_program():
    global _CACHED_NC
    if _CACHED_NC is None:
        nc = bacc.Bacc(
            "TRN2",
            target_bir_lowering=False,
            debug=False,
            enable_asserts=False,
        )
        with tile.TileContext(nc) as tc:
            _dli_body(tc)
        nc.compile()
        _CACHED_NC = nc
    return _CACHED_NC


def _make_in_maps(inputs):
    enc = np.asarray(inputs["encoder_output"], dtype=np.float32)
    mask = np.ascontiguousarray(inputs["mask"], dtype=np.int32)
    w_e = np.asarray(inputs["fc_w"], dtype=np.float32)[0, H:]
    # one-hot expanded matvec weights: woh[e, BPC*b + m] = w[e] * (m == b)
    woh = np.zeros((E, BPC * BPC), dtype=ml_dtypes.bfloat16)
    woh[:, :: BPC + 1] = w_e[:, None].astype(ml_dtypes.bfloat16)
    woh = np.ascontiguousarray(woh)
    maps = []
    for i in range(NCORES):
        # [BPC, T, E] -> [E, BPC*T], bf16 (same RNE cast as the on-device one)
        shard = enc[i * BPC : (i + 1) * BPC].transpose(2, 0, 1).reshape(E, BPC * T)
        maps.append(
            {
                "enc": np.ascontiguousarray(shard).astype(ml_dtypes.bfloat16),
                "woh": woh,
                "mask": np.ascontiguousarray(mask[i * BPC : (i + 1) * BPC]),
            }
        )
    return maps


def _finalize(results):
    numer = sum(float(r["out"][0]) for r in results)
    denom = sum(float(r["out"][1]) for r in results)
    return np.asarray(numer / denom, dtype=np.float32)


def kernel(**inputs) -> np.ndarray:
    nc = _get_program()
    res = run_bass_kernel_spmd(nc, _make_in_maps(inputs), list(range(NCORES)))
    return _finalize(res.results)


# revision 4
# speedup vs baseline: 1.2608x; 1.2608x over previous
"""
DLI loss kernel for Trainium2 (8 NeuronCores, pure data parallel over batch).

Math
----
The reference computes, per (b, j) window pair:
    logits[b,j,k] = h_last[b,j]@w_h + cterm[b,k] + fc_b
    loss_pair     = LSE_k(logits masked to k in [j+3, len_b)) - logits[b,j,j+3]
The h_last@w_h and fc_b terms are constant in k, so they cancel exactly
between the LSE and the positive logit.  The whole LSTM drops out and

    loss = sum_{b, s in [3, len_b)} [ log(sum_{k=s}^{len_b-1} e^{cterm[b,k]})
                                      - cterm[b,s] ] / sum_b (len_b - 3)
    cterm[b,k] = encoder_output[b,k,:] @ fc_w[0, H:]   (valid region only)

cterm values are O(+-2) so no max-subtraction is needed for a stable exp.

Device pipeline (per core, 16 batch rows)
-----------------------------------------
The host marshals enc into a per-core [E, BPC*T] layout in fp8-e4m3
(measured loss rel-err 2.7e-05 on the fixed seed, vs the 2e-2 gate), so E
lands on partitions and each DMA chunk is 128 clean 2 KB descriptors.  It
also pre-expands fc_w into the one-hot bf16 matvec weight matrix woh
(woh[e, 16b + m] = w[e] * (m == b)).

  - 4 enc DMAs (4 rows each) on the sync HWDGE queue; woh + mask ride the
    scalar queue.
  - PE/DVE/ACT run a few dummy warm-up ops during the DMA stream so DVFS
    has the engines at full clock when the real work arrives.
  - 16 accumulating PE matvecs (lhsT = woh column block, rhs = enc slice)
    compute cterm for all 16 rows into one PSUM tile [16, 512].
  - Tail: exp (ACT, fused time-reverse via the PSUM read AP),
    tensor_tensor_scan (suffix sums with mask fold), Ln(x+1) with
    accumulate, masked-cterm accumulate; per-row [numer-parts, denom]
    go straight to HBM ([16, 3]) and the host reduces.
"""

import ml_dtypes
import numpy as np

import concourse.bacc as bacc
import concourse.bass as bass
import concourse.mybir as mybir
import concourse.tile as tile
from concourse._compat import with_exitstack
from concourse.bass_utils import run_bass_kernel_spmd

B, T, E, H = 128, 512, 128, 128
NCORES = 8
BPC = B // NCORES  # batch rows per core
RPD = 4  # rows per DMA chunk
NCHUNK = BPC // RPD

f32 = mybir.dt.float32
bf16 = mybir.dt.bfloat16
fp8 = mybir.dt.float8e4
i32 = mybir.dt.int32
u16 = mybir.dt.uint16

NPEWARM = 7
NDVEWARM = 4
NACTWARM = 2


@with_exitstack
def _dli_body(ctx, tc):
    nc = tc.nc

    enc = nc.dram_tensor("enc", [E, BPC * T], fp8, kind="ExternalInput").ap()
    woh_d = nc.dram_tensor("woh", [E, BPC * BPC], bf16, kind="ExternalInput").ap()
    msk = nc.dram_tensor("mask", [BPC, T], i32, kind="ExternalInput").ap()
    out = nc.dram_tensor("out", [BPC, 2], f32, kind="ExternalOutput").ap()

    const_pool = ctx.enter_context(tc.tile_pool(name="const", bufs=1))
    ct_psum = ctx.enter_context(tc.tile_pool(name="ct_psum", bufs=1, space="PSUM"))
    wm_psum = ctx.enter_context(tc.tile_pool(name="wm_psum", bufs=1, space="PSUM"))
    sc_pool = ctx.enter_context(tc.tile_pool(name="scan", bufs=1))

    # woh and mask ride the (otherwise idle) scalar HWDGE queue so the
    # matvecs and the tail never wait behind the enc stream.
    woh = const_pool.tile([E, BPC * BPC], bf16)
    nc.scalar.dma_start(woh[:], woh_d[:, :])
    msk_sb = sc_pool.tile([BPC, T], i32)
    nc.scalar.dma_start(msk_sb[:], msk[:, :])

    # enc stream: 4 rows per DMA (128 x 2 KB descriptors), sync queue.
    enc_sb = const_pool.tile([E, BPC * T], fp8)
    CW = RPD * T
    for q in range(NCHUNK):
        nc.sync.dma_start(
            enc_sb[:, q * CW : (q + 1) * CW], enc[:, q * CW : (q + 1) * CW]
        )

    # engine warm-up during the DMA stream: DVFS needs sustained activity
    # to clock the engines up, and the first real op otherwise runs 2-3x
    # slow.  All dummies work on scratch tiles with no data dependencies.
    scr = const_pool.tile([E, T], bf16)
    nc.vector.memset(scr[:].bitcast(u16), 16256)  # bf16 1.0
    scr2 = const_pool.tile([E, T], f32)
    dummy_ps = wm_psum.tile([E, T], f32)
    for _ in range(NPEWARM):
        nc.tensor.matmul(
            dummy_ps[:, :], lhsT=scr[:, 0:E], rhs=scr[:, :], start=True, stop=True
        )
    for _ in range(NDVEWARM):
        nc.vector.tensor_copy(scr2[:], scr[:])
    for _ in range(NACTWARM):
        nc.scalar.activation(scr2[:], scr[:], mybir.ActivationFunctionType.Exp)

    # cterm for all 16 rows accumulated in one PSUM tile; free index = t.
    cterm_ps = ct_psum.tile([BPC, T], f32)
    for b in range(BPC):
        nc.tensor.matmul(
            cterm_ps[:, :],
            lhsT=woh[:, BPC * b : BPC * (b + 1)],
            rhs=enc_sb[:, b * T : (b + 1) * T],
            start=(b == 0),
            stop=(b == BPC - 1),
        )

    # mask -> f32, zero first 3 time steps (window starts need s >= 3)
    maskf = sc_pool.tile([BPC, T], f32)
    nc.vector.tensor_copy(maskf[:], msk_sb[:])
    nc.vector.memset(maskf[:, 0:3], 0.0)
    mask3_rev = maskf[:, ::-1]

    # denominator: sum(mask3) = len_b - 3 (mask-only, runs during the stream)
    acc0 = sc_pool.tile([BPC, 1], f32)
    acc1 = sc_pool.tile([BPC, 1], f32)
    packed = sc_pool.tile([BPC, 2], f32)
    nc.vector.tensor_reduce(
        packed[:, 1:2], maskf[:], axis=mybir.AxisListType.X, op=mybir.AluOpType.add
    )

    # E = exp(cterm), time-reversed via the PSUM read AP
    e_sb = sc_pool.tile([BPC, T], f32)
    nc.scalar.activation(e_sb[:], cterm_ps[:, ::-1], mybir.ActivationFunctionType.Exp)

    # suffix sums with the mask folded into the scan:
    # state = (E[i] + state) * mask3_rev[i] - resets across the invalid
    # tail, accumulates sum(exp) over the valid region.
    s_sb = sc_pool.tile([BPC, T], f32)
    nc.vector.tensor_tensor_scan(
        s_sb[:], e_sb[:], mask3_rev, 0.0, mybir.AluOpType.add, mybir.AluOpType.mult
    )

    # u = (S - 1) * mask3; then ln(u + 1) = log(S) on valid, 0 on invalid
    u_sb = sc_pool.tile([BPC, T], f32)
    nc.vector.scalar_tensor_tensor(
        u_sb[:], s_sb[:], 1.0, mask3_rev,
        mybir.AluOpType.subtract, mybir.AluOpType.mult,
    )
    ln_sb = sc_pool.tile([BPC, T], f32)
    nc.scalar.activation(
        ln_sb[:], u_sb[:], mybir.ActivationFunctionType.Ln,
        bias=1.0, scale=1.0, accum_out=acc0[:, 0:1],
    )

    # sum(mask3*cterm): needs only the matvec results; emitted after the
    # scan/stt so it overlaps the ACT Ln instead of blocking the DVE chain.
    mc_sb = sc_pool.tile([BPC, T], f32)
    nc.vector.scalar_tensor_tensor(
        mc_sb[:], cterm_ps[:, :], 0.0, maskf[:],
        mybir.AluOpType.add, mybir.AluOpType.mult, accum_out=acc1[:, 0:1],
    )

    # packed[:,0] = numer_b = sum(ln) - sum(mask3*cterm); host sums rows.
    nc.vector.tensor_tensor(
        packed[:, 0:1], acc0[:, 0:1], acc1[:, 0:1], mybir.AluOpType.subtract
    )
    nc.scalar.dma_start(out[:, :], packed[:])


_CACHED_NC = None


def _get_program():
    global _CACHED_NC
    if _CACHED_NC is None:
        nc = bacc.Bacc(
            "TRN2",
            target_bir_lowering=False,
            debug=False,
            enable_asserts=False,
        )
        with tile.TileContext(nc) as tc:
            _dli_body(tc)
        nc.compile()
        _CACHED_NC = nc
    return _CACHED_NC


def _make_in_maps(inputs):
    enc = np.asarray(inputs["encoder_output"], dtype=np.float32)
    mask = np.ascontiguousarray(inputs["mask"], dtype=np.int32)
    w_e = np.asarray(inputs["fc_w"], dtype=np.float32)[0, H:]
    # one-hot expanded matvec weights: woh[e, BPC*b + m] = w[e] * (m == b)
    woh = np.zeros((E, BPC * BPC), dtype=ml_dtypes.bfloat16)
    woh[:, :: BPC + 1] = w_e[:, None].astype(ml_dtypes.bfloat16)
    woh = np.ascontiguousarray(woh)
    maps = []
    for i in range(NCORES):
        # [BPC, T, E] -> [E, BPC*T], fp8 e4m3
        shard = enc[i * BPC : (i + 1) * BPC].transpose(2, 0, 1).reshape(E, BPC * T)
        maps.append(
            {
                "enc": np.ascontiguousarray(shard).astype(ml_dtypes.float8_e4m3),
                "woh": woh,
                "mask": np.ascontiguousarray(mask[i * BPC : (i + 1) * BPC]),
            }
        )
    return maps


def _finalize(results):
    numer = sum(float(r["out"][:, 0].sum()) for r in results)
    denom = sum(float(r["out"][:, 1].sum()) for r in results)
    return np.asarray(numer / denom, dtype=np.float32)


def kernel(**inputs) -> np.ndarray:
    nc = _get_program()
    res = run_bass_kernel_spmd(nc, _make_in_maps(inputs), list(range(NCORES)))
    return _finalize(res.results)
